# revision 24
# baseline (speedup 1.0000x reference)
"""Trainium2 Bass kernel for nn_Attention (dense transformer spatial attention).

Reference computation (per batch b):
    q = Wq @ x   (1x1 conv over channels), k = Wk @ c, v = Wv @ c
    per head h (8 heads, head_dim 32, n = 64*64 = 4096 tokens):
        S = (q_h^T k_h) * DIM**-0.5 ; P = softmax(S, axis=-1) ; o_h = v_h P^T
    out = Wo @ concat(o_h)

Sharding (8 cores): core c handles batch b = c//2 and heads 4*(c%2) .. +4
(tensor-parallel over heads via weight row/col slicing).  The two cores of a
batch produce partial outputs Y = Wo_slice @ o_slice which the host sums.

v2 design (vs the 565us baseline): the kernel is engine-bound on the softmax
exp (every score element must cross ACT/DVE once at 1 elem/cycle/lane fp32),
so the structure keeps both exp engines saturated and the PE dense:

  - i-blocks of 512 tokens (8 blocks), j-chunks of 128 keys (32/block).
  - Scores: per chunk, 4 row-tiled matmuls (K=32 per head at PE rows
    32h) run concurrently, writing two [128, 1024] fp32 PSUM pair-tiles
    (2 heads side by side, 2 banks each) from a 3-deep ring (6 banks).
    One wide exp op per pair-tile (ACT table exp or DVE Schraudolph
    bit-trick), deficit-scheduled across the two engines.
  - AV: lags 2 chunks behind scores; per chunk 4 col-tiled matmuls
    (M=33: 32 v-dims + ones column for the softmax denominator) at PE
    cols 0/64, accumulating into per-head-pair PSUM tiles (2 banks).
  - Normalize: ACT copies acc->SBUF, DVE reciprocal of the denominator
    rows, GpSimd broadcast + multiply (keeps the exp engines free).
  - Output projection rides the score PSUM ring (2 matmuls per
    pair-tile slot), evacuated by ACT/DVE and DMA'd out.
  - The steady chunk cadence keeps PE gaps well under the ~3.4us HAM
    window, so the PE clock stays at 2.4 GHz (the baseline oscillated,
    spending 54% of its time throttled at 1.2 GHz).
"""

import os
import sys

import numpy as np

for _p in ("/opt/trn_rl_repo", "/root/.axon_site/_ro/trn_rl_repo"):
    if os.path.isdir(_p) and _p not in sys.path:
        sys.path.insert(0, _p)

import concourse.bass as bass
import concourse.tile as tile
from concourse import bacc, mybir
from concourse.bass import ts
from concourse.bass_utils import run_bass_kernel_spmd
from concourse.masks import make_identity

DIM = 512
HEAD = 8
ATTN_DIM = 256
HEAD_DIM = 32
N = 4096  # 64 * 64 tokens
SCALE = DIM ** -0.5

N_CORES = 8
NI = 512   # i-block (query tokens per block)
NJ = 128   # j-chunk (key tokens per score matmul)
N_IB = N // NI   # 8 i-blocks
N_JC = N // NJ   # 32 j-chunks
AV_LAG = 5       # AV trails the score cursor by this many chunks.  Deep
                 # enough that a standing queue of ready AV chunks spans
                 # each block boundary: the per-block first-AV delay (acc
                 # bank handoff) then never leaves the PE with score-only
                 # slots, which would trip the HAM idle window and drop
                 # the PE clock to 1.2 GHz for ~20us.

F32 = mybir.dt.float32
F16 = mybir.dt.float16
I16 = mybir.dt.int16

# Schraudolph fast-exp on the Vector engine: the bit pattern of fp16
# exp(x) is approximately int16(x * 1024/ln2 + 15*1024 + sigma).  One
# tensor_scalar (mult, add) with an int16 output view computes it in a
# single instruction; rel err ~N(0, 1.8%) which washes out over the
# 4096-way diffuse softmax.
EXP_A = 1024.0 / float(np.log(2.0))
EXP_B = 15.0 * 1024.0 - 60.0

# Deficit scheduler estimates (us per [128, 1024] exp op, trace-measured).
DVE_EXP_US = 1.222
ACT_EXP_US = 1.072


def build_nc():
    nc = bacc.Bacc()

    x_d = nc.dram_tensor("x", [DIM, N], F16, kind="ExternalInput").ap()
    c_d = nc.dram_tensor("c", [DIM, N], F16, kind="ExternalInput").ap()
    wqt_d = nc.dram_tensor("wqt", [DIM, 128], F16, kind="ExternalInput").ap()
    wkt_d = nc.dram_tensor("wkt", [DIM, 128], F16, kind="ExternalInput").ap()
    wvt_d = nc.dram_tensor("wvt", [DIM, 128], F16, kind="ExternalInput").ap()
    wot_d = nc.dram_tensor("wot", [128, DIM], F16, kind="ExternalInput").ap()
    y_d = nc.dram_tensor("y", [DIM, N], F32, kind="ExternalOutput").ap()

    from contextlib import ExitStack

    with tile.TileContext(nc) as tc, ExitStack() as stk:
        persist = stk.enter_context(tc.tile_pool(name="persist", bufs=1))

        q_sb = persist.tile([128, N], F16)
        k_sb = persist.tile([128, N], F16)
        # vT: (token-in-chunk, j_chunk, head, 32 dims + ones col)
        vT_sb = persist.tile([128, N_JC, 4, HEAD_DIM + 1], F16)
        wot_sb = persist.tile([128, DIM], F16)
        ident = persist.tile([128, 128], F16)

        nc.sync.dma_start(out=wot_sb, in_=wot_d)
        make_identity(nc, ident)
        nc.vector.memset(vT_sb[:, :, :, HEAD_DIM:], 1.0)

        # Preload the exp activation table set during the DMA lead-in so
        # the first real exp doesn't pay the ~2.7us ACT_TABLE_LOAD.
        warm_sb = persist.tile([1, 32], F32)
        nc.vector.memset(warm_sb, 0.0)
        nc.scalar.activation(out=warm_sb, in_=warm_sb,
                             func=mybir.ActivationFunctionType.Exp)

        # Round-robin small PSUM->SBUF evacuations over Vector and Scalar.
        _cp_state = {"n": 0}

        def copy_rr(out, in_):
            _cp_state["n"] += 1
            if _cp_state["n"] % 2:
                nc.vector.tensor_copy(out=out, in_=in_)
            else:
                nc.scalar.copy(out=out, in_=in_)

        # Exp-engine deficit scheduler (us of queued work per engine).
        eng_t = {"D": 0.0, "A": 0.0}

        def pick_exp_engine():
            if eng_t["D"] + DVE_EXP_US <= eng_t["A"] + ACT_EXP_US:
                eng_t["D"] += DVE_EXP_US
                return "D"
            eng_t["A"] += ACT_EXP_US
            return "A"

        # ---------------- Phase 1: projections ----------------
        cw_pool = stk.enter_context(tc.tile_pool(name="cw", bufs=1))
        v_sb = cw_pool.tile([128, N], F16, tag="v_sb")

        # Score/proj/outproj PSUM ring: 3 x [128, 1024] fp32 (6 banks).
        sc_ps = stk.enter_context(tc.tile_pool(name="sc_ps", bufs=3,
                                               space="PSUM"))
        # AV accumulators: heads (0,1) and (2,3), [*, 512] fp32 (2 banks).
        av_ps = stk.enter_context(tc.tile_pool(name="av_ps", bufs=1,
                                               space="PSUM"))

        with tc.tile_pool(name="xc_in", bufs=1) as xc_pool:
            w_sb = {}
            for nm, d in (("wkt", wkt_d), ("wqt", wqt_d), ("wvt", wvt_d)):
                w = xc_pool.tile([128, 4, 128], F16, tag=nm)
                nc.sync.dma_start(out=w, in_=d.rearrange("(c p) m -> p c m", p=128))
                w_sb[nm] = w
            # Load in token-quarters so the first projection matmuls wait
            # on 1MB of context instead of 4MB.
            c_t = [xc_pool.tile([128, N], F16, tag="c_in", bufs=4,
                                name=f"c_in_{cc}") for cc in range(4)]
            x_t = [xc_pool.tile([128, N], F16, tag="x_in", bufs=4,
                                name=f"x_in_{cc}") for cc in range(4)]
            for q in range(4):
                for cc in range(4):
                    nc.sync.dma_start(out=c_t[cc][:, ts(q, N // 4)],
                                      in_=c_d[ts(cc, 128), ts(q, N // 4)])
            for q in range(4):
                for cc in range(4):
                    nc.gpsimd.dma_start(out=x_t[cc][:, ts(q, N // 4)],
                                        in_=x_d[ts(cc, 128), ts(q, N // 4)])

            # Pre-warm the PE clock during the DMA lead-in: ~5us of dummy
            # matmuls un-throttle HAM (4/8 -> 8/8) so the projections run
            # at 2.4 GHz from their first instruction.
            warm_ps = sc_ps.tile([128, 128], F32, tag="sc", name="warm_ps")
            for _ in range(48):
                nc.tensor.matmul(warm_ps, lhsT=ident, rhs=ident,
                                 start=True, stop=True)

            def project(wname, src, dst):
                w = w_sb[wname]
                for half in range(N // 1024):  # 4 slots of 2 n-tiles
                    ps = sc_ps.tile([128, 1024], F32, tag="sc",
                                    name=f"pj_{wname}_{half}")
                    for sub in range(2):
                        t = 2 * half + sub
                        for cc in range(4):
                            nc.tensor.matmul(
                                ps[:, ts(sub, NI)],
                                lhsT=w[:, cc, :], rhs=src[cc][:, ts(t, NI)],
                                start=(cc == 0), stop=(cc == 3),
                            )
                    copy_rr(out=dst[:, ts(half, 1024)], in_=ps)

            project("wkt", c_t, k_sb)
            project("wqt", x_t, q_sb)
            project("wvt", c_t, v_sb)

        # V transposes: vT[j-chunk] = v[:, chunk].T, PE transpose via ident.
        for ch in range(N_JC):
            tp = sc_ps.tile([128, 128], F16, tag="sc", name=f"vtp_{ch}")
            nc.tensor.transpose(tp, v_sb[:, ts(ch, 128)], ident)
            copy_rr(
                out=vT_sb[:, ch, :, 0:HEAD_DIM],
                in_=tp.rearrange("p (h d) -> p h d", h=4),
            )

        # ---------------- Phase 2: attention ----------------
        at_pool = stk.enter_context(tc.tile_pool(name="attn", bufs=1))

        es_ref = {}      # (ib, j) -> es pair-tile list [pair01, pair23]
        acc_ref = {}     # ib -> (accA, accB)
        raw_ref = {}     # ib -> raw tile
        pending_norm = []  # per-head normalize closures, drained 1/chunk
        norm_left = {}   # ib -> heads still to normalize
        pending_op = []  # i-blocks awaiting output projection

        def emit_scores(ib, j):
            """4 row-tiled score MMs -> 2 psum pair-tiles; exp to SBUF."""
            pairs = []
            for p in range(2):  # heads (2p, 2p+1)
                ps = sc_ps.tile([128, 1024], F32, tag="sc",
                                name=f"scps_{ib}_{j}_{p}")
                es = at_pool.tile([128, 1024], F16, tag=f"es{p}", bufs=9,
                                  name=f"es_{ib}_{j}_{p}")
                for hh in range(2):
                    h = 2 * p + hh
                    base = 32 * h
                    nc.tensor.matmul(
                        ps[:, ts(hh, NI)],
                        lhsT=k_sb[base:base + 32, ts(j, NJ)],
                        rhs=q_sb[base:base + 32, ts(ib, NI)],
                        start=True, stop=True,
                        tile_position=(base, 0),
                    )
                eng = pick_exp_engine()
                if eng == "A":
                    nc.scalar.activation(
                        out=es, in_=ps,
                        func=mybir.ActivationFunctionType.Exp,
                    )
                else:
                    nc.vector.tensor_scalar(
                        out=es.bitcast(I16), in0=ps,
                        scalar1=EXP_A, scalar2=EXP_B,
                        op0=mybir.AluOpType.mult,
                        op1=mybir.AluOpType.add,
                    )
                pairs.append(es)
            es_ref[(ib, j)] = pairs

        def emit_av(ib, j):
            if j == 0:
                accA = av_ps.tile([128, NI], F32, tag="accA",
                                  name=f"accA_{ib}")
                accB = av_ps.tile([128, NI], F32, tag="accB",
                                  name=f"accB_{ib}")
                acc_ref[ib] = (accA, accB)
            accA, accB = acc_ref[ib]
            pairs = es_ref.pop((ib, j))
            for p in range(2):
                acc = (accA, accB)[p]
                es = pairs[p]
                for hh in range(2):
                    nc.tensor.matmul(
                        acc[64 * hh:64 * hh + HEAD_DIM + 1, :],
                        lhsT=vT_sb[:, j, 2 * p + hh, :],
                        rhs=es[:, ts(hh, NI)],
                        start=(j == 0), stop=(j == N_JC - 1),
                        tile_position=(0, 64 * hh),
                        skip_group_check=True,
                    )
            if j == N_JC - 1:
                emit_normalize(ib)

        def emit_normalize(ib):
            # Free the acc banks quickly (2 ACT copies), then hand the
            # per-head reciprocal-normalize to GpSimd (DVE only does the
            # tiny reciprocal).  The per-head tails are drained one per
            # chunk by the main loop so neither exp engine sees a burst.
            accA, accB = acc_ref.pop(ib)
            raw = at_pool.tile([128, NI], F16, tag="raw", bufs=2,
                               name=f"raw_{ib}")
            raw_ref[ib] = raw
            norm_left[ib] = 4
            nsbs = []
            for p, acc in enumerate((accA, accB)):
                nsb = at_pool.tile([128, NI], F32, tag=f"nsb{p}", bufs=2,
                                   name=f"nsb_{ib}_{p}")
                # Split across engines so the acc banks free ASAP (the
                # next block's first AV matmul waits on them).
                if eng_t["D"] + 0.6 <= eng_t["A"] + 0.72:
                    nc.vector.tensor_copy(out=nsb, in_=acc)
                    eng_t["D"] += 0.6
                else:
                    nc.scalar.copy(out=nsb, in_=acc)
                    eng_t["A"] += 0.72
                nsbs.append(nsb)

            def make_tail(h):
                p, hh = divmod(h, 2)
                nsb = nsbs[p]

                def tail():
                    lr = at_pool.tile([1, NI], F32, tag="lr", bufs=4,
                                      name=f"lr_{ib}_{h}")
                    rc = at_pool.tile([1, NI], F32, tag="rc", bufs=4,
                                      name=f"rc_{ib}_{h}")
                    bc = at_pool.tile([128, NI], F32, tag="bc", bufs=4,
                                      name=f"bc_{ib}_{h}")
                    nc.vector.tensor_copy(
                        out=lr,
                        in_=nsb[64 * hh + HEAD_DIM:64 * hh + HEAD_DIM + 1, :])
                    nc.vector.reciprocal_approx_fast(out=rc, in_=lr)
                    eng_t["D"] += 0.6
                    # Full-partition broadcast so the SB*SB multiply sees
                    # equal base partitions on both inputs.  The multiply
                    # stays on DVE: mixing gpsimd op types (broadcast +
                    # mul) thrashes its microcode library (~6us per swap).
                    nc.gpsimd.partition_broadcast(bc, rc)
                    nc.vector.tensor_mul(
                        out=raw[ts(h, 32), :],
                        in0=nsb[64 * hh:64 * hh + 32, :],
                        in1=bc[64 * hh:64 * hh + 32, :],
                    )
                    eng_t["D"] += 0.6
                    norm_left[ib] -= 1
                    if norm_left[ib] == 0:
                        del norm_left[ib]
                        pending_op.append(ib)

                return tail

            for h in range(4):
                pending_norm.append(make_tail(h))

        def emit_outproj():
            while pending_op:
                oi = pending_op.pop(0)
                raw = raw_ref.pop(oi)
                for half in range(2):  # cc pairs (0,1) and (2,3)
                    ps = sc_ps.tile([128, 1024], F32, tag="sc",
                                    name=f"ofps_{oi}_{half}")
                    for sub in range(2):
                        cc = 2 * half + sub
                        nc.tensor.matmul(
                            ps[:, ts(sub, NI)],
                            lhsT=wot_sb[:, ts(cc, 128)], rhs=raw,
                            start=True, stop=True,
                        )
                    ot = at_pool.tile([128, 1024], F32, tag="ot", bufs=3,
                                      name=f"ot_{oi}_{half}")
                    copy_rr(out=ot, in_=ps)
                    eng_t["A" if _cp_state["n"] % 2 == 0 else "D"] += 1.2
                    for sub in range(2):
                        cc = 2 * half + sub
                        nc.sync.dma_start(
                            out=y_d[ts(cc, 128), ts(oi, NI)],
                            in_=ot[:, ts(sub, NI)])

        # Main pipeline: score cursor leads the AV cursor by AV_LAG chunks.
        total = N_IB * N_JC
        av_cur = 0
        for s_cur in range(total):
            ib, j = divmod(s_cur, N_JC)
            emit_scores(ib, j)
            # One normalize tail every 4th chunk: keeps the DVE's extra
            # duty ~0.3us/chunk so AV's exps never queue behind it.  In
            # the last block drain every other chunk so the final output
            # projection isn't pushed past the end of the score stream.
            if pending_norm and s_cur % (2 if ib == N_IB - 1 else 4) == 1:
                pending_norm.pop(0)()
            if pending_op:
                emit_outproj()
            navail = 0
            while av_cur <= s_cur - AV_LAG and navail < 2:
                av_ib, av_j = divmod(av_cur, N_JC)
                # Delay each block's first AV a few extra chunks so the
                # PE never queues behind the normalize copies that free
                # the acc banks (keeps HAM warm across block boundaries).
                if av_j == 0 and s_cur < av_ib * N_JC + 4:
                    break
                emit_av(av_ib, av_j)
                av_cur += 1
                navail += 1
        while av_cur < total:
            emit_av(*divmod(av_cur, N_JC))
            av_cur += 1
        while pending_norm:
            pending_norm.pop(0)()
        emit_outproj()

    nc.compile()
    return nc


_NC_CACHE = None


def _get_nc():
    global _NC_CACHE
    if _NC_CACHE is None:
        _NC_CACHE = build_nc()
    return _NC_CACHE


def _shard_inputs(query, context, Wq, Wk, Wv, Wo):
    query = np.asarray(query, dtype=np.float32)
    context = np.asarray(context, dtype=np.float32)
    Wq = np.asarray(Wq, dtype=np.float32)
    Wk = np.asarray(Wk, dtype=np.float32)
    Wv = np.asarray(Wv, dtype=np.float32)
    Wo = np.asarray(Wo, dtype=np.float32)
    b = query.shape[0]
    in_maps = []
    for core in range(N_CORES):
        bb, p = divmod(core, 2)
        sl = slice(128 * p, 128 * (p + 1))
        in_maps.append({
            "x": np.ascontiguousarray(query[bb].reshape(DIM, N).astype(np.float16)),
            "c": np.ascontiguousarray(context[bb].reshape(DIM, N).astype(np.float16)),
            "wqt": np.ascontiguousarray((Wq[sl, :] * SCALE).T.astype(np.float16)),
            "wkt": np.ascontiguousarray(Wk[sl, :].T.astype(np.float16)),
            "wvt": np.ascontiguousarray(Wv[sl, :].T.astype(np.float16)),
            "wot": np.ascontiguousarray(Wo[:, sl].T.astype(np.float16)),
        })
    return in_maps, b


def _run(inputs, trace=False, **kw):
    in_maps, b = _shard_inputs(**inputs)
    nc = _get_nc()
    res = run_bass_kernel_spmd(nc, in_maps, core_ids=list(range(N_CORES)),
                               trace=trace, **kw)
    outs = []
    for bb in range(b):
        y = res.results[2 * bb]["y"] + res.results[2 * bb + 1]["y"]
        outs.append(y.reshape(DIM, 64, 64))
    return np.stack(outs).astype(np.float32), res


def kernel(**inputs):
    out, _ = _run(inputs)
    return out


# revision 26
# speedup vs baseline: 1.1601x; 1.1601x over previous
"""Trainium2 Bass kernel for nn_Attention (dense transformer spatial attention).

Reference computation (per batch b):
    q = Wq @ x   (1x1 conv over channels), k = Wk @ c, v = Wv @ c
    per head h (8 heads, head_dim 32, n = 64*64 = 4096 tokens):
        S = (q_h^T k_h) * DIM**-0.5 ; P = softmax(S, axis=-1) ; o_h = v_h P^T
    out = Wo @ concat(o_h)

Sharding (8 cores): core c handles batch b = c//2 and heads 4*(c%2) .. +4
(tensor-parallel over heads via weight row/col slicing).  The two cores of a
batch produce partial outputs Y = Wo_slice @ o_slice which the host sums.

v2 design (vs the 565us baseline): the kernel is engine-bound on the softmax
exp (every score element must cross ACT/DVE once at 1 elem/cycle/lane fp32),
so the structure keeps both exp engines saturated and the PE dense:

  - i-blocks of 512 tokens (8 blocks), j-chunks of 128 keys (32/block).
  - Scores: per chunk, 4 row-tiled matmuls (K=32 per head at PE rows
    32h) run concurrently, writing two [128, 1024] fp32 PSUM pair-tiles
    (2 heads side by side, 2 banks each) from a 3-deep ring (6 banks).
    One wide exp op per pair-tile (ACT table exp or DVE Schraudolph
    bit-trick), deficit-scheduled across the two engines.
  - AV: lags 2 chunks behind scores; per chunk 4 col-tiled matmuls
    (M=33: 32 v-dims + ones column for the softmax denominator) at PE
    cols 0/64, accumulating into per-head-pair PSUM tiles (2 banks).
  - Normalize: ACT copies acc->SBUF, DVE reciprocal of the denominator
    rows, GpSimd broadcast + multiply (keeps the exp engines free).
  - Output projection rides the score PSUM ring (2 matmuls per
    pair-tile slot), evacuated by ACT/DVE and DMA'd out.
  - The steady chunk cadence keeps PE gaps well under the ~3.4us HAM
    window, so the PE clock stays at 2.4 GHz (the baseline oscillated,
    spending 54% of its time throttled at 1.2 GHz).
"""

import os
import sys

import numpy as np

for _p in ("/opt/trn_rl_repo", "/root/.axon_site/_ro/trn_rl_repo"):
    if os.path.isdir(_p) and _p not in sys.path:
        sys.path.insert(0, _p)

import concourse.bass as bass
import concourse.tile as tile
from concourse import bacc, mybir
from concourse.bass import ts
from concourse.bass_utils import run_bass_kernel_spmd
from concourse.masks import make_identity

DIM = 512
HEAD = 8
ATTN_DIM = 256
HEAD_DIM = 32
N = 4096  # 64 * 64 tokens
SCALE = DIM ** -0.5

N_CORES = 8
NI = 512   # i-block (query tokens per block)
NJ = 128   # j-chunk (key tokens per score matmul)
N_IB = N // NI   # 8 i-blocks
N_JC = N // NJ   # 32 j-chunks
AV_LAG = 3       # AV trails the score cursor by this many chunks

F32 = mybir.dt.float32
F16 = mybir.dt.float16
I16 = mybir.dt.int16

# Schraudolph fast-exp on the Vector engine: the bit pattern of fp16
# exp(x) is approximately int16(x * 1024/ln2 + 15*1024 + sigma).  One
# tensor_scalar (mult, add) with an int16 output view computes it in a
# single instruction; rel err ~N(0, 1.8%) which washes out over the
# 4096-way diffuse softmax.
EXP_A = 1024.0 / float(np.log(2.0))
EXP_B = 15.0 * 1024.0 - 60.0

# Deficit scheduler estimates (us per [128, 1024] exp op, trace-measured).
DVE_EXP_US = 1.222
ACT_EXP_US = 1.072


def build_nc():
    nc = bacc.Bacc()

    x_d = nc.dram_tensor("x", [DIM, N], F16, kind="ExternalInput").ap()
    c_d = nc.dram_tensor("c", [DIM, N], F16, kind="ExternalInput").ap()
    wqt_d = nc.dram_tensor("wqt", [DIM, 128], F16, kind="ExternalInput").ap()
    wkt_d = nc.dram_tensor("wkt", [DIM, 128], F16, kind="ExternalInput").ap()
    wvt_d = nc.dram_tensor("wvt", [DIM, 128], F16, kind="ExternalInput").ap()
    wot_d = nc.dram_tensor("wot", [128, DIM], F16, kind="ExternalInput").ap()
    y_d = nc.dram_tensor("y", [DIM, N], F32, kind="ExternalOutput").ap()

    from contextlib import ExitStack

    with tile.TileContext(nc) as tc, ExitStack() as stk:
        persist = stk.enter_context(tc.tile_pool(name="persist", bufs=1))

        q_sb = persist.tile([128, N], F16)
        k_sb = persist.tile([128, N], F16)
        # vT: (token-in-chunk, j_chunk, head, 32 dims + ones col)
        vT_sb = persist.tile([128, N_JC, 4, HEAD_DIM + 1], F16)
        wot_sb = persist.tile([128, DIM], F16)
        ident = persist.tile([128, 128], F16)

        nc.sync.dma_start(out=wot_sb, in_=wot_d)
        make_identity(nc, ident)
        nc.vector.memset(vT_sb[:, :, :, HEAD_DIM:], 1.0)

        # Preload the exp activation table set during the DMA lead-in so
        # the first real exp doesn't pay the ~2.7us ACT_TABLE_LOAD.
        warm_sb = persist.tile([1, 32], F32)
        nc.vector.memset(warm_sb, 0.0)
        nc.scalar.activation(out=warm_sb, in_=warm_sb,
                             func=mybir.ActivationFunctionType.Exp)

        # Round-robin small PSUM->SBUF evacuations over Vector and Scalar.
        _cp_state = {"n": 0}

        def copy_rr(out, in_):
            _cp_state["n"] += 1
            if _cp_state["n"] % 2:
                nc.vector.tensor_copy(out=out, in_=in_)
            else:
                nc.scalar.copy(out=out, in_=in_)

        # Exp-engine deficit scheduler (us of queued work per engine).
        eng_t = {"D": 0.0, "A": 0.0}

        def pick_exp_engine():
            if eng_t["D"] + DVE_EXP_US <= eng_t["A"] + ACT_EXP_US:
                eng_t["D"] += DVE_EXP_US
                return "D"
            eng_t["A"] += ACT_EXP_US
            return "A"

        # ---------------- Phase 1: projections ----------------
        cw_pool = stk.enter_context(tc.tile_pool(name="cw", bufs=1))
        v_sb = cw_pool.tile([128, N], F16, tag="v_sb")

        # Score/proj/outproj PSUM ring: 3 x [128, 1024] fp32 (6 banks).
        sc_ps = stk.enter_context(tc.tile_pool(name="sc_ps", bufs=3,
                                               space="PSUM"))
        # AV accumulators: heads (0,1) and (2,3), [*, 512] fp32 (2 banks).
        av_ps = stk.enter_context(tc.tile_pool(name="av_ps", bufs=1,
                                               space="PSUM"))

        with tc.tile_pool(name="xc_in", bufs=1) as xc_pool:
            w_sb = {}
            for nm, d in (("wkt", wkt_d), ("wqt", wqt_d), ("wvt", wvt_d)):
                w = xc_pool.tile([128, 4, 128], F16, tag=nm)
                nc.sync.dma_start(out=w, in_=d.rearrange("(c p) m -> p c m", p=128))
                w_sb[nm] = w
            c_t = []
            for cc in range(4):
                t = xc_pool.tile([128, N], F16, tag="c_in", bufs=4)
                nc.sync.dma_start(out=t, in_=c_d[ts(cc, 128), :])
                c_t.append(t)
            x_t = []
            for cc in range(4):
                t = xc_pool.tile([128, N], F16, tag="x_in", bufs=4)
                nc.gpsimd.dma_start(out=t, in_=x_d[ts(cc, 128), :])
                x_t.append(t)

            # Pre-warm the PE clock during the DMA lead-in: ~5us of dummy
            # matmuls un-throttle HAM (4/8 -> 8/8) so the projections run
            # at 2.4 GHz from their first instruction.  One accumulation
            # group, so Tile adds no inter-matmul semaphores.
            NWARM = 44
            warm_ps = sc_ps.tile([128, 128], F32, tag="sc", name="warm_ps")
            for wi in range(NWARM):
                nc.tensor.matmul(warm_ps, lhsT=ident, rhs=ident,
                                 start=(wi == 0), stop=(wi == NWARM - 1))

            def project(wname, src, dst):
                w = w_sb[wname]
                for half in range(N // 1024):  # 4 slots of 2 n-tiles
                    ps = sc_ps.tile([128, 1024], F32, tag="sc",
                                    name=f"pj_{wname}_{half}")
                    for sub in range(2):
                        t = 2 * half + sub
                        for cc in range(4):
                            nc.tensor.matmul(
                                ps[:, ts(sub, NI)],
                                lhsT=w[:, cc, :], rhs=src[cc][:, ts(t, NI)],
                                start=(cc == 0), stop=(cc == 3),
                            )
                    copy_rr(out=dst[:, ts(half, 1024)], in_=ps)

            project("wkt", c_t, k_sb)
            project("wqt", x_t, q_sb)
            project("wvt", c_t, v_sb)

        # V transposes: vT[j-chunk] = v[:, chunk].T, PE transpose via ident.
        for ch in range(N_JC):
            tp = sc_ps.tile([128, 128], F16, tag="sc", name=f"vtp_{ch}")
            nc.tensor.transpose(tp, v_sb[:, ts(ch, 128)], ident)
            copy_rr(
                out=vT_sb[:, ch, :, 0:HEAD_DIM],
                in_=tp.rearrange("p (h d) -> p h d", h=4),
            )

        # ---------------- Phase 2: attention ----------------
        at_pool = stk.enter_context(tc.tile_pool(name="attn", bufs=1))

        es_ref = {}      # (ib, j) -> es pair-tile list [pair01, pair23]
        acc_ref = {}     # ib -> (accA, accB)
        raw_ref = {}     # ib -> raw tile
        pending_norm = []  # per-head normalize closures, drained 1/chunk
        norm_left = {}   # ib -> heads still to normalize
        pending_op = []  # i-blocks awaiting output projection

        def emit_scores(ib, j):
            """4 row-tiled score MMs -> 2 psum pair-tiles; exp to SBUF."""
            pairs = []
            for p in range(2):  # heads (2p, 2p+1)
                ps = sc_ps.tile([128, 1024], F32, tag="sc",
                                name=f"scps_{ib}_{j}_{p}")
                es = at_pool.tile([128, 1024], F16, tag=f"es{p}", bufs=6,
                                  name=f"es_{ib}_{j}_{p}")
                for hh in range(2):
                    h = 2 * p + hh
                    base = 32 * h
                    nc.tensor.matmul(
                        ps[:, ts(hh, NI)],
                        lhsT=k_sb[base:base + 32, ts(j, NJ)],
                        rhs=q_sb[base:base + 32, ts(ib, NI)],
                        start=True, stop=True,
                        tile_position=(base, 0),
                    )
                eng = pick_exp_engine()
                if eng == "A":
                    nc.scalar.activation(
                        out=es, in_=ps,
                        func=mybir.ActivationFunctionType.Exp,
                    )
                else:
                    nc.vector.tensor_scalar(
                        out=es.bitcast(I16), in0=ps,
                        scalar1=EXP_A, scalar2=EXP_B,
                        op0=mybir.AluOpType.mult,
                        op1=mybir.AluOpType.add,
                    )
                pairs.append(es)
            es_ref[(ib, j)] = pairs

        def emit_av(ib, j):
            if j == 0:
                accA = av_ps.tile([128, NI], F32, tag="accA",
                                  name=f"accA_{ib}")
                accB = av_ps.tile([128, NI], F32, tag="accB",
                                  name=f"accB_{ib}")
                acc_ref[ib] = (accA, accB)
            accA, accB = acc_ref[ib]
            pairs = es_ref.pop((ib, j))
            for p in range(2):
                acc = (accA, accB)[p]
                es = pairs[p]
                for hh in range(2):
                    nc.tensor.matmul(
                        acc[64 * hh:64 * hh + HEAD_DIM + 1, :],
                        lhsT=vT_sb[:, j, 2 * p + hh, :],
                        rhs=es[:, ts(hh, NI)],
                        start=(j == 0), stop=(j == N_JC - 1),
                        tile_position=(0, 64 * hh),
                        skip_group_check=True,
                    )
            if j == N_JC - 1:
                emit_normalize(ib)

        def emit_normalize(ib):
            # Free the acc banks quickly (2 ACT copies), then hand the
            # per-head reciprocal-normalize to GpSimd (DVE only does the
            # tiny reciprocal).  The per-head tails are drained one per
            # chunk by the main loop so neither exp engine sees a burst.
            accA, accB = acc_ref.pop(ib)
            raw = at_pool.tile([128, NI], F16, tag="raw", bufs=2,
                               name=f"raw_{ib}")
            raw_ref[ib] = raw
            norm_left[ib] = 4
            nsbs = []
            for p, acc in enumerate((accA, accB)):
                nsb = at_pool.tile([128, NI], F32, tag=f"nsb{p}", bufs=2,
                                   name=f"nsb_{ib}_{p}")
                # Split across engines so the acc banks free ASAP (the
                # next block's first AV matmul waits on them).
                if eng_t["D"] + 0.6 <= eng_t["A"] + 0.72:
                    nc.vector.tensor_copy(out=nsb, in_=acc)
                    eng_t["D"] += 0.6
                else:
                    nc.scalar.copy(out=nsb, in_=acc)
                    eng_t["A"] += 0.72
                nsbs.append(nsb)

            def make_tail(h):
                p, hh = divmod(h, 2)
                nsb = nsbs[p]

                def tail():
                    lr = at_pool.tile([1, NI], F32, tag="lr", bufs=4,
                                      name=f"lr_{ib}_{h}")
                    rc = at_pool.tile([1, NI], F32, tag="rc", bufs=4,
                                      name=f"rc_{ib}_{h}")
                    bc = at_pool.tile([128, NI], F32, tag="bc", bufs=4,
                                      name=f"bc_{ib}_{h}")
                    nc.vector.tensor_copy(
                        out=lr,
                        in_=nsb[64 * hh + HEAD_DIM:64 * hh + HEAD_DIM + 1, :])
                    nc.vector.reciprocal_approx_fast(out=rc, in_=lr)
                    eng_t["D"] += 0.6
                    # Full-partition broadcast so the SB*SB multiply sees
                    # equal base partitions on both inputs.  The multiply
                    # stays on DVE: mixing gpsimd op types (broadcast +
                    # mul) thrashes its microcode library (~6us per swap).
                    nc.gpsimd.partition_broadcast(bc, rc)
                    nc.vector.tensor_mul(
                        out=raw[ts(h, 32), :],
                        in0=nsb[64 * hh:64 * hh + 32, :],
                        in1=bc[64 * hh:64 * hh + 32, :],
                    )
                    eng_t["D"] += 0.6
                    norm_left[ib] -= 1
                    if norm_left[ib] == 0:
                        del norm_left[ib]
                        pending_op.append(ib)

                return tail

            for h in range(4):
                pending_norm.append(make_tail(h))

        def emit_outproj():
            while pending_op:
                oi = pending_op.pop(0)
                raw = raw_ref.pop(oi)
                for half in range(2):  # cc pairs (0,1) and (2,3)
                    ps = sc_ps.tile([128, 1024], F32, tag="sc",
                                    name=f"ofps_{oi}_{half}")
                    for sub in range(2):
                        cc = 2 * half + sub
                        nc.tensor.matmul(
                            ps[:, ts(sub, NI)],
                            lhsT=wot_sb[:, ts(cc, 128)], rhs=raw,
                            start=True, stop=True,
                        )
                    ot = at_pool.tile([128, 1024], F32, tag="ot", bufs=3,
                                      name=f"ot_{oi}_{half}")
                    copy_rr(out=ot, in_=ps)
                    eng_t["A" if _cp_state["n"] % 2 == 0 else "D"] += 1.2
                    for sub in range(2):
                        cc = 2 * half + sub
                        nc.sync.dma_start(
                            out=y_d[ts(cc, 128), ts(oi, NI)],
                            in_=ot[:, ts(sub, NI)])

        # Main pipeline: score cursor leads the AV cursor by AV_LAG chunks.
        total = N_IB * N_JC
        av_cur = 0
        for s_cur in range(total):
            ib, j = divmod(s_cur, N_JC)
            emit_scores(ib, j)
            # One normalize tail every 4th chunk: keeps the DVE's extra
            # duty ~0.3us/chunk so AV's exps never queue behind it.
            if pending_norm and s_cur % 4 == 1:
                pending_norm.pop(0)()
            if pending_op:
                emit_outproj()
            navail = 0
            while av_cur <= s_cur - AV_LAG and navail < 2:
                av_ib, av_j = divmod(av_cur, N_JC)
                # Delay each block's first AV a few extra chunks so the
                # PE never queues behind the normalize copies that free
                # the acc banks (keeps HAM warm across block boundaries).
                if av_j == 0 and s_cur < av_ib * N_JC + 4:
                    break
                emit_av(av_ib, av_j)
                av_cur += 1
                navail += 1
        while av_cur < total:
            emit_av(*divmod(av_cur, N_JC))
            av_cur += 1
        while pending_norm:
            pending_norm.pop(0)()
        emit_outproj()

    nc.compile()
    return nc


_NC_CACHE = None


def _get_nc():
    global _NC_CACHE
    if _NC_CACHE is None:
        _NC_CACHE = build_nc()
    return _NC_CACHE


def _shard_inputs(query, context, Wq, Wk, Wv, Wo):
    query = np.asarray(query, dtype=np.float32)
    context = np.asarray(context, dtype=np.float32)
    Wq = np.asarray(Wq, dtype=np.float32)
    Wk = np.asarray(Wk, dtype=np.float32)
    Wv = np.asarray(Wv, dtype=np.float32)
    Wo = np.asarray(Wo, dtype=np.float32)
    b = query.shape[0]
    in_maps = []
    for core in range(N_CORES):
        bb, p = divmod(core, 2)
        sl = slice(128 * p, 128 * (p + 1))
        in_maps.append({
            "x": np.ascontiguousarray(query[bb].reshape(DIM, N).astype(np.float16)),
            "c": np.ascontiguousarray(context[bb].reshape(DIM, N).astype(np.float16)),
            "wqt": np.ascontiguousarray((Wq[sl, :] * SCALE).T.astype(np.float16)),
            "wkt": np.ascontiguousarray(Wk[sl, :].T.astype(np.float16)),
            "wvt": np.ascontiguousarray(Wv[sl, :].T.astype(np.float16)),
            "wot": np.ascontiguousarray(Wo[:, sl].T.astype(np.float16)),
        })
    return in_maps, b


def _run(inputs, trace=False, **kw):
    in_maps, b = _shard_inputs(**inputs)
    nc = _get_nc()
    res = run_bass_kernel_spmd(nc, in_maps, core_ids=list(range(N_CORES)),
                               trace=trace, **kw)
    outs = []
    for bb in range(b):
        y = res.results[2 * bb]["y"] + res.results[2 * bb + 1]["y"]
        outs.append(y.reshape(DIM, 64, 64))
    return np.stack(outs).astype(np.float32), res


def kernel(**inputs):
    out, _ = _run(inputs)
    return out


# revision 27
# speedup vs baseline: 1.1720x; 1.0102x over previous
"""Trainium2 Bass kernel for nn_Attention (dense transformer spatial attention).

Reference computation (per batch b):
    q = Wq @ x   (1x1 conv over channels), k = Wk @ c, v = Wv @ c
    per head h (8 heads, head_dim 32, n = 64*64 = 4096 tokens):
        S = (q_h^T k_h) * DIM**-0.5 ; P = softmax(S, axis=-1) ; o_h = v_h P^T
    out = Wo @ concat(o_h)

Sharding (8 cores): core c handles batch b = c//2 and heads 4*(c%2) .. +4
(tensor-parallel over heads via weight row/col slicing).  The two cores of a
batch produce partial outputs Y = Wo_slice @ o_slice which the host sums.

v2 design (vs the 565us baseline): the kernel is engine-bound on the softmax
exp (every score element must cross ACT/DVE once at 1 elem/cycle/lane fp32),
so the structure keeps both exp engines saturated and the PE dense:

  - i-blocks of 512 tokens (8 blocks), j-chunks of 128 keys (32/block).
  - Scores: per chunk, 4 row-tiled matmuls (K=32 per head at PE rows
    32h) run concurrently, writing two [128, 1024] fp32 PSUM pair-tiles
    (2 heads side by side, 2 banks each) from a 3-deep ring (6 banks).
    One wide exp op per pair-tile (ACT table exp or DVE Schraudolph
    bit-trick), deficit-scheduled across the two engines.
  - AV: lags 2 chunks behind scores; per chunk 4 col-tiled matmuls
    (M=33: 32 v-dims + ones column for the softmax denominator) at PE
    cols 0/64, accumulating into per-head-pair PSUM tiles (2 banks).
  - Normalize: ACT copies acc->SBUF, DVE reciprocal of the denominator
    rows, GpSimd broadcast + multiply (keeps the exp engines free).
  - Output projection rides the score PSUM ring (2 matmuls per
    pair-tile slot), evacuated by ACT/DVE and DMA'd out.
  - The steady chunk cadence keeps PE gaps well under the ~3.4us HAM
    window, so the PE clock stays at 2.4 GHz (the baseline oscillated,
    spending 54% of its time throttled at 1.2 GHz).
"""

import os
import sys

import numpy as np

for _p in ("/opt/trn_rl_repo", "/root/.axon_site/_ro/trn_rl_repo"):
    if os.path.isdir(_p) and _p not in sys.path:
        sys.path.insert(0, _p)

import concourse.bass as bass
import concourse.tile as tile
from concourse import bacc, mybir
from concourse.bass import ts
from concourse.bass_utils import run_bass_kernel_spmd
from concourse.masks import make_identity

DIM = 512
HEAD = 8
ATTN_DIM = 256
HEAD_DIM = 32
N = 4096  # 64 * 64 tokens
SCALE = DIM ** -0.5

N_CORES = 8
NI = 512   # i-block (query tokens per block)
NJ = 128   # j-chunk (key tokens per score matmul)
N_IB = N // NI   # 8 i-blocks
N_JC = N // NJ   # 32 j-chunks
AV_LAG = 5       # AV trails the score cursor by this many chunks.  Deep
                 # enough that a standing queue of ready AV chunks spans
                 # each block boundary, so the acc-bank handoff never
                 # leaves the PE with score-only slots (HAM idle window).

F32 = mybir.dt.float32
F16 = mybir.dt.float16
I16 = mybir.dt.int16

# Schraudolph fast-exp on the Vector engine: the bit pattern of fp16
# exp(x) is approximately int16(x * 1024/ln2 + 15*1024 + sigma).  One
# tensor_scalar (mult, add) with an int16 output view computes it in a
# single instruction; rel err ~N(0, 1.8%) which washes out over the
# 4096-way diffuse softmax.
EXP_A = 1024.0 / float(np.log(2.0))
EXP_B = 15.0 * 1024.0 - 60.0

# Deficit scheduler estimates (us per [128, 1024] exp op, trace-measured).
DVE_EXP_US = 1.222
ACT_EXP_US = 1.072


def build_nc():
    nc = bacc.Bacc()

    x_d = nc.dram_tensor("x", [DIM, N], F16, kind="ExternalInput").ap()
    c_d = nc.dram_tensor("c", [DIM, N], F16, kind="ExternalInput").ap()
    wqt_d = nc.dram_tensor("wqt", [DIM, 128], F16, kind="ExternalInput").ap()
    wkt_d = nc.dram_tensor("wkt", [DIM, 128], F16, kind="ExternalInput").ap()
    wvt_d = nc.dram_tensor("wvt", [DIM, 128], F16, kind="ExternalInput").ap()
    wot_d = nc.dram_tensor("wot", [128, DIM], F16, kind="ExternalInput").ap()
    y_d = nc.dram_tensor("y", [DIM, N], F32, kind="ExternalOutput").ap()

    from contextlib import ExitStack

    with tile.TileContext(nc) as tc, ExitStack() as stk:
        persist = stk.enter_context(tc.tile_pool(name="persist", bufs=1))

        q_sb = persist.tile([128, N], F16)
        k_sb = persist.tile([128, N], F16)
        # vT: (token-in-chunk, j_chunk, head, 32 dims + ones col)
        vT_sb = persist.tile([128, N_JC, 4, HEAD_DIM + 1], F16)
        wot_sb = persist.tile([128, DIM], F16)
        ident = persist.tile([128, 128], F16)

        nc.sync.dma_start(out=wot_sb, in_=wot_d)
        make_identity(nc, ident)
        nc.vector.memset(vT_sb[:, :, :, HEAD_DIM:], 1.0)

        # Preload the exp activation table set during the DMA lead-in so
        # the first real exp doesn't pay the ~2.7us ACT_TABLE_LOAD.
        warm_sb = persist.tile([1, 32], F32)
        nc.vector.memset(warm_sb, 0.0)
        nc.scalar.activation(out=warm_sb, in_=warm_sb,
                             func=mybir.ActivationFunctionType.Exp)

        # Round-robin small PSUM->SBUF evacuations over Vector and Scalar.
        _cp_state = {"n": 0}

        def copy_rr(out, in_):
            _cp_state["n"] += 1
            if _cp_state["n"] % 2:
                nc.vector.tensor_copy(out=out, in_=in_)
            else:
                nc.scalar.copy(out=out, in_=in_)

        # Exp-engine deficit scheduler (us of queued work per engine).
        eng_t = {"D": 0.0, "A": 0.0}

        def pick_exp_engine():
            if eng_t["D"] + DVE_EXP_US <= eng_t["A"] + ACT_EXP_US:
                eng_t["D"] += DVE_EXP_US
                return "D"
            eng_t["A"] += ACT_EXP_US
            return "A"

        # ---------------- Phase 1: projections ----------------
        cw_pool = stk.enter_context(tc.tile_pool(name="cw", bufs=1))
        v_sb = cw_pool.tile([128, N], F16, tag="v_sb")

        # Score/proj/outproj PSUM ring: 3 x [128, 1024] fp32 (6 banks).
        sc_ps = stk.enter_context(tc.tile_pool(name="sc_ps", bufs=3,
                                               space="PSUM"))
        # AV accumulators: heads (0,1) and (2,3), [*, 512] fp32 (2 banks).
        av_ps = stk.enter_context(tc.tile_pool(name="av_ps", bufs=1,
                                               space="PSUM"))

        with tc.tile_pool(name="xc_in", bufs=1) as xc_pool:
            w_sb = {}
            for nm, d in (("wkt", wkt_d), ("wqt", wqt_d), ("wvt", wvt_d)):
                w = xc_pool.tile([128, 4, 128], F16, tag=nm)
                nc.sync.dma_start(out=w, in_=d.rearrange("(c p) m -> p c m", p=128))
                w_sb[nm] = w
            # Token-quarter loads: the first projection matmuls wait on
            # 1MB of context instead of 4MB.
            c_t = [xc_pool.tile([128, N], F16, tag="c_in", bufs=4,
                                name=f"c_in_{cc}") for cc in range(4)]
            x_t = [xc_pool.tile([128, N], F16, tag="x_in", bufs=4,
                                name=f"x_in_{cc}") for cc in range(4)]
            for q in range(4):
                for cc in range(4):
                    nc.sync.dma_start(out=c_t[cc][:, ts(q, N // 4)],
                                      in_=c_d[ts(cc, 128), ts(q, N // 4)])
            for q in range(4):
                for cc in range(4):
                    nc.gpsimd.dma_start(out=x_t[cc][:, ts(q, N // 4)],
                                        in_=x_d[ts(cc, 128), ts(q, N // 4)])

            # Pre-warm the PE clock during the DMA lead-in: ~5us of dummy
            # matmuls un-throttle HAM (4/8 -> 8/8) so the projections run
            # at 2.4 GHz from their first instruction.  One accumulation
            # group, so Tile adds no inter-matmul semaphores.
            NWARM = 150
            warm_ps = sc_ps.tile([128, 128], F32, tag="sc", name="warm_ps")
            for wi in range(NWARM):
                nc.tensor.matmul(warm_ps, lhsT=ident, rhs=ident,
                                 start=(wi == 0), stop=(wi == NWARM - 1))

            def project(wname, src, dst):
                w = w_sb[wname]
                for half in range(N // 1024):  # 4 slots of 2 n-tiles
                    ps = sc_ps.tile([128, 1024], F32, tag="sc",
                                    name=f"pj_{wname}_{half}")
                    for sub in range(2):
                        t = 2 * half + sub
                        for cc in range(4):
                            nc.tensor.matmul(
                                ps[:, ts(sub, NI)],
                                lhsT=w[:, cc, :], rhs=src[cc][:, ts(t, NI)],
                                start=(cc == 0), stop=(cc == 3),
                            )
                    copy_rr(out=dst[:, ts(half, 1024)], in_=ps)

            project("wkt", c_t, k_sb)
            project("wqt", x_t, q_sb)
            project("wvt", c_t, v_sb)

        # V transposes: vT[j-chunk] = v[:, chunk].T, PE transpose via ident.
        for ch in range(N_JC):
            tp = sc_ps.tile([128, 128], F16, tag="sc", name=f"vtp_{ch}")
            nc.tensor.transpose(tp, v_sb[:, ts(ch, 128)], ident)
            copy_rr(
                out=vT_sb[:, ch, :, 0:HEAD_DIM],
                in_=tp.rearrange("p (h d) -> p h d", h=4),
            )

        # ---------------- Phase 2: attention ----------------
        at_pool = stk.enter_context(tc.tile_pool(name="attn", bufs=1))

        es_ref = {}      # (ib, j) -> es pair-tile list [pair01, pair23]
        acc_ref = {}     # ib -> (accA, accB)
        raw_ref = {}     # ib -> raw tile
        pending_norm = []  # per-head normalize closures, drained 1/chunk
        norm_left = {}   # ib -> heads still to normalize
        pending_op = []  # i-blocks awaiting output projection

        def emit_scores(ib, j):
            """4 row-tiled score MMs -> 2 psum pair-tiles; exp to SBUF."""
            pairs = []
            for p in range(2):  # heads (2p, 2p+1)
                ps = sc_ps.tile([128, 1024], F32, tag="sc",
                                name=f"scps_{ib}_{j}_{p}")
                es = at_pool.tile([128, 1024], F16, tag=f"es{p}", bufs=9,
                                  name=f"es_{ib}_{j}_{p}")
                for hh in range(2):
                    h = 2 * p + hh
                    base = 32 * h
                    nc.tensor.matmul(
                        ps[:, ts(hh, NI)],
                        lhsT=k_sb[base:base + 32, ts(j, NJ)],
                        rhs=q_sb[base:base + 32, ts(ib, NI)],
                        start=True, stop=True,
                        tile_position=(base, 0),
                    )
                eng = pick_exp_engine()
                if eng == "A":
                    nc.scalar.activation(
                        out=es, in_=ps,
                        func=mybir.ActivationFunctionType.Exp,
                    )
                else:
                    nc.vector.tensor_scalar(
                        out=es.bitcast(I16), in0=ps,
                        scalar1=EXP_A, scalar2=EXP_B,
                        op0=mybir.AluOpType.mult,
                        op1=mybir.AluOpType.add,
                    )
                pairs.append(es)
            es_ref[(ib, j)] = pairs

        def emit_av(ib, j):
            if j == 0:
                accA = av_ps.tile([128, NI], F32, tag="accA",
                                  name=f"accA_{ib}")
                accB = av_ps.tile([128, NI], F32, tag="accB",
                                  name=f"accB_{ib}")
                acc_ref[ib] = (accA, accB)
            accA, accB = acc_ref[ib]
            pairs = es_ref.pop((ib, j))
            for p in range(2):
                acc = (accA, accB)[p]
                es = pairs[p]
                for hh in range(2):
                    nc.tensor.matmul(
                        acc[64 * hh:64 * hh + HEAD_DIM + 1, :],
                        lhsT=vT_sb[:, j, 2 * p + hh, :],
                        rhs=es[:, ts(hh, NI)],
                        start=(j == 0), stop=(j == N_JC - 1),
                        tile_position=(0, 64 * hh),
                        skip_group_check=True,
                    )
            if j == N_JC - 1:
                emit_normalize(ib)

        def emit_normalize(ib):
            # Free the acc banks quickly (2 ACT copies), then hand the
            # per-head reciprocal-normalize to GpSimd (DVE only does the
            # tiny reciprocal).  The per-head tails are drained one per
            # chunk by the main loop so neither exp engine sees a burst.
            accA, accB = acc_ref.pop(ib)
            raw = at_pool.tile([128, NI], F16, tag="raw", bufs=2,
                               name=f"raw_{ib}")
            raw_ref[ib] = raw
            norm_left[ib] = 4
            nsbs = []
            for p, acc in enumerate((accA, accB)):
                nsb = at_pool.tile([128, NI], F32, tag=f"nsb{p}", bufs=2,
                                   name=f"nsb_{ib}_{p}")
                # Split across engines so the acc banks free ASAP (the
                # next block's first AV matmul waits on them).
                if eng_t["D"] + 0.6 <= eng_t["A"] + 0.72:
                    nc.vector.tensor_copy(out=nsb, in_=acc)
                    eng_t["D"] += 0.6
                else:
                    nc.scalar.copy(out=nsb, in_=acc)
                    eng_t["A"] += 0.72
                nsbs.append(nsb)

            def make_tail(h):
                p, hh = divmod(h, 2)
                nsb = nsbs[p]

                def tail():
                    lr = at_pool.tile([1, NI], F32, tag="lr", bufs=4,
                                      name=f"lr_{ib}_{h}")
                    rc = at_pool.tile([1, NI], F32, tag="rc", bufs=4,
                                      name=f"rc_{ib}_{h}")
                    bc = at_pool.tile([128, NI], F32, tag="bc", bufs=4,
                                      name=f"bc_{ib}_{h}")
                    nc.vector.tensor_copy(
                        out=lr,
                        in_=nsb[64 * hh + HEAD_DIM:64 * hh + HEAD_DIM + 1, :])
                    nc.vector.reciprocal_approx_fast(out=rc, in_=lr)
                    eng_t["D"] += 0.6
                    # Full-partition broadcast so the SB*SB multiply sees
                    # equal base partitions on both inputs.  The multiply
                    # stays on DVE: mixing gpsimd op types (broadcast +
                    # mul) thrashes its microcode library (~6us per swap).
                    nc.gpsimd.partition_broadcast(bc, rc)
                    nc.vector.tensor_mul(
                        out=raw[ts(h, 32), :],
                        in0=nsb[64 * hh:64 * hh + 32, :],
                        in1=bc[64 * hh:64 * hh + 32, :],
                    )
                    eng_t["D"] += 0.6
                    norm_left[ib] -= 1
                    if norm_left[ib] == 0:
                        del norm_left[ib]
                        pending_op.append(ib)

                return tail

            for h in range(4):
                pending_norm.append(make_tail(h))

        def emit_outproj():
            while pending_op:
                oi = pending_op.pop(0)
                raw = raw_ref.pop(oi)
                for half in range(2):  # cc pairs (0,1) and (2,3)
                    ps = sc_ps.tile([128, 1024], F32, tag="sc",
                                    name=f"ofps_{oi}_{half}")
                    for sub in range(2):
                        cc = 2 * half + sub
                        nc.tensor.matmul(
                            ps[:, ts(sub, NI)],
                            lhsT=wot_sb[:, ts(cc, 128)], rhs=raw,
                            start=True, stop=True,
                        )
                    ot = at_pool.tile([128, 1024], F32, tag="ot", bufs=3,
                                      name=f"ot_{oi}_{half}")
                    copy_rr(out=ot, in_=ps)
                    eng_t["A" if _cp_state["n"] % 2 == 0 else "D"] += 1.2
                    for sub in range(2):
                        cc = 2 * half + sub
                        nc.sync.dma_start(
                            out=y_d[ts(cc, 128), ts(oi, NI)],
                            in_=ot[:, ts(sub, NI)])

        # Main pipeline: score cursor leads the AV cursor by AV_LAG chunks.
        total = N_IB * N_JC
        av_cur = 0
        for s_cur in range(total):
            ib, j = divmod(s_cur, N_JC)
            emit_scores(ib, j)
            # One normalize tail every 4th chunk: keeps the DVE's extra
            # duty ~0.3us/chunk so AV's exps never queue behind it.
            if pending_norm and s_cur % 4 == 1:
                pending_norm.pop(0)()
            if pending_op:
                emit_outproj()
            navail = 0
            while av_cur <= s_cur - AV_LAG and navail < 2:
                av_ib, av_j = divmod(av_cur, N_JC)
                # Delay each block's first AV a few extra chunks so the
                # PE never queues behind the normalize copies that free
                # the acc banks (keeps HAM warm across block boundaries).
                if av_j == 0 and s_cur < av_ib * N_JC + 4:
                    break
                emit_av(av_ib, av_j)
                av_cur += 1
                navail += 1
        while av_cur < total:
            emit_av(*divmod(av_cur, N_JC))
            av_cur += 1
        while pending_norm:
            pending_norm.pop(0)()
        emit_outproj()

    nc.compile()
    return nc


_NC_CACHE = None


def _get_nc():
    global _NC_CACHE
    if _NC_CACHE is None:
        _NC_CACHE = build_nc()
    return _NC_CACHE


def _shard_inputs(query, context, Wq, Wk, Wv, Wo):
    query = np.asarray(query, dtype=np.float32)
    context = np.asarray(context, dtype=np.float32)
    Wq = np.asarray(Wq, dtype=np.float32)
    Wk = np.asarray(Wk, dtype=np.float32)
    Wv = np.asarray(Wv, dtype=np.float32)
    Wo = np.asarray(Wo, dtype=np.float32)
    b = query.shape[0]
    in_maps = []
    for core in range(N_CORES):
        bb, p = divmod(core, 2)
        sl = slice(128 * p, 128 * (p + 1))
        in_maps.append({
            "x": np.ascontiguousarray(query[bb].reshape(DIM, N).astype(np.float16)),
            "c": np.ascontiguousarray(context[bb].reshape(DIM, N).astype(np.float16)),
            "wqt": np.ascontiguousarray((Wq[sl, :] * SCALE).T.astype(np.float16)),
            "wkt": np.ascontiguousarray(Wk[sl, :].T.astype(np.float16)),
            "wvt": np.ascontiguousarray(Wv[sl, :].T.astype(np.float16)),
            "wot": np.ascontiguousarray(Wo[:, sl].T.astype(np.float16)),
        })
    return in_maps, b


def _run(inputs, trace=False, **kw):
    in_maps, b = _shard_inputs(**inputs)
    nc = _get_nc()
    res = run_bass_kernel_spmd(nc, in_maps, core_ids=list(range(N_CORES)),
                               trace=trace, **kw)
    outs = []
    for bb in range(b):
        y = res.results[2 * bb]["y"] + res.results[2 * bb + 1]["y"]
        outs.append(y.reshape(DIM, 64, 64))
    return np.stack(outs).astype(np.float32), res


def kernel(**inputs):
    out, _ = _run(inputs)
    return out


# revision 28
# speedup vs baseline: 1.1795x; 1.0065x over previous
"""Trainium2 Bass kernel for nn_Attention (dense transformer spatial attention).

Reference computation (per batch b):
    q = Wq @ x   (1x1 conv over channels), k = Wk @ c, v = Wv @ c
    per head h (8 heads, head_dim 32, n = 64*64 = 4096 tokens):
        S = (q_h^T k_h) * DIM**-0.5 ; P = softmax(S, axis=-1) ; o_h = v_h P^T
    out = Wo @ concat(o_h)

Sharding (8 cores): core c handles batch b = c//2 and heads 4*(c%2) .. +4
(tensor-parallel over heads via weight row/col slicing).  The two cores of a
batch produce partial outputs Y = Wo_slice @ o_slice which the host sums.

v2 design (vs the 565us baseline): the kernel is engine-bound on the softmax
exp (every score element must cross ACT/DVE once at 1 elem/cycle/lane fp32),
so the structure keeps both exp engines saturated and the PE dense:

  - i-blocks of 512 tokens (8 blocks), j-chunks of 128 keys (32/block).
  - Scores: per chunk, 4 row-tiled matmuls (K=32 per head at PE rows
    32h) run concurrently, writing two [128, 1024] fp32 PSUM pair-tiles
    (2 heads side by side, 2 banks each) from a 3-deep ring (6 banks).
    One wide exp op per pair-tile (ACT table exp or DVE Schraudolph
    bit-trick), deficit-scheduled across the two engines.
  - AV: lags 2 chunks behind scores; per chunk 4 col-tiled matmuls
    (M=33: 32 v-dims + ones column for the softmax denominator) at PE
    cols 0/64, accumulating into per-head-pair PSUM tiles (2 banks).
  - Normalize: ACT copies acc->SBUF, DVE reciprocal of the denominator
    rows, GpSimd broadcast + multiply (keeps the exp engines free).
  - Output projection rides the score PSUM ring (2 matmuls per
    pair-tile slot), evacuated by ACT/DVE and DMA'd out.
  - The steady chunk cadence keeps PE gaps well under the ~3.4us HAM
    window, so the PE clock stays at 2.4 GHz (the baseline oscillated,
    spending 54% of its time throttled at 1.2 GHz).
"""

import os
import sys

import numpy as np

for _p in ("/opt/trn_rl_repo", "/root/.axon_site/_ro/trn_rl_repo"):
    if os.path.isdir(_p) and _p not in sys.path:
        sys.path.insert(0, _p)

import concourse.bass as bass
import concourse.tile as tile
from concourse import bacc, mybir
from concourse.bass import ts
from concourse.bass_utils import run_bass_kernel_spmd
from concourse.masks import make_identity

DIM = 512
HEAD = 8
ATTN_DIM = 256
HEAD_DIM = 32
N = 4096  # 64 * 64 tokens
SCALE = DIM ** -0.5

N_CORES = 8
NI = 512   # i-block (query tokens per block)
NJ = 128   # j-chunk (key tokens per score matmul)
N_IB = N // NI   # 8 i-blocks
N_JC = N // NJ   # 32 j-chunks
AV_LAG = 5       # AV trails the score cursor by this many chunks.  Deep
                 # enough that a standing queue of ready AV chunks spans
                 # each block boundary, so the acc-bank handoff never
                 # leaves the PE with score-only slots (HAM idle window).

F32 = mybir.dt.float32
F16 = mybir.dt.float16
I16 = mybir.dt.int16

# Schraudolph fast-exp on the Vector engine: the bit pattern of fp16
# exp(x) is approximately int16(x * 1024/ln2 + 15*1024 + sigma).  One
# tensor_scalar (mult, add) with an int16 output view computes it in a
# single instruction; rel err ~N(0, 1.8%) which washes out over the
# 4096-way diffuse softmax.
EXP_A = 1024.0 / float(np.log(2.0))
EXP_B = 15.0 * 1024.0 - 60.0

# Deficit scheduler estimates (us per [128, 1024] exp op, trace-measured).
DVE_EXP_US = 1.222
ACT_EXP_US = 1.072


def build_nc():
    nc = bacc.Bacc()

    x_d = nc.dram_tensor("x", [DIM, N], F16, kind="ExternalInput").ap()
    c_d = nc.dram_tensor("c", [DIM, N], F16, kind="ExternalInput").ap()
    wqt_d = nc.dram_tensor("wqt", [DIM, 128], F16, kind="ExternalInput").ap()
    wkt_d = nc.dram_tensor("wkt", [DIM, 128], F16, kind="ExternalInput").ap()
    wvt_d = nc.dram_tensor("wvt", [DIM, 128], F16, kind="ExternalInput").ap()
    wot_d = nc.dram_tensor("wot", [128, DIM], F16, kind="ExternalInput").ap()
    y_d = nc.dram_tensor("y", [DIM, N], F32, kind="ExternalOutput").ap()

    from contextlib import ExitStack

    with tile.TileContext(nc) as tc, ExitStack() as stk:
        persist = stk.enter_context(tc.tile_pool(name="persist", bufs=1))

        q_sb = persist.tile([128, N], F16)
        k_sb = persist.tile([128, N], F16)
        # vT: (token-in-chunk, j_chunk, head, 32 dims + ones col)
        vT_sb = persist.tile([128, N_JC, 4, HEAD_DIM + 1], F16)
        wot_sb = persist.tile([128, DIM], F16)
        ident = persist.tile([128, 128], F16)

        nc.sync.dma_start(out=wot_sb, in_=wot_d)
        make_identity(nc, ident)
        nc.vector.memset(vT_sb[:, :, :, HEAD_DIM:], 1.0)

        # Preload the exp activation table set during the DMA lead-in so
        # the first real exp doesn't pay the ~2.7us ACT_TABLE_LOAD.
        warm_sb = persist.tile([1, 32], F32)
        nc.vector.memset(warm_sb, 0.0)
        nc.scalar.activation(out=warm_sb, in_=warm_sb,
                             func=mybir.ActivationFunctionType.Exp)

        # Round-robin small PSUM->SBUF evacuations over Vector and Scalar.
        _cp_state = {"n": 0}

        def copy_rr(out, in_):
            _cp_state["n"] += 1
            if _cp_state["n"] % 2:
                nc.vector.tensor_copy(out=out, in_=in_)
            else:
                nc.scalar.copy(out=out, in_=in_)

        # Exp-engine deficit scheduler (us of queued work per engine).
        eng_t = {"D": 0.0, "A": 0.0}

        def pick_exp_engine():
            if eng_t["D"] + DVE_EXP_US <= eng_t["A"] + ACT_EXP_US:
                eng_t["D"] += DVE_EXP_US
                return "D"
            eng_t["A"] += ACT_EXP_US
            return "A"

        # ---------------- Phase 1: projections ----------------
        cw_pool = stk.enter_context(tc.tile_pool(name="cw", bufs=1))
        v_sb = cw_pool.tile([128, N], F16, tag="v_sb")

        # Score/proj/outproj PSUM ring: 3 x [128, 1024] fp32 (6 banks).
        sc_ps = stk.enter_context(tc.tile_pool(name="sc_ps", bufs=3,
                                               space="PSUM"))
        # AV accumulators: heads (0,1) and (2,3), [*, 512] fp32 (2 banks).
        av_ps = stk.enter_context(tc.tile_pool(name="av_ps", bufs=1,
                                               space="PSUM"))

        with tc.tile_pool(name="xc_in", bufs=1) as xc_pool:
            w_sb = {}
            for nm, d in (("wkt", wkt_d), ("wqt", wqt_d), ("wvt", wvt_d)):
                w = xc_pool.tile([128, 4, 128], F16, tag=nm)
                nc.sync.dma_start(out=w, in_=d.rearrange("(c p) m -> p c m", p=128))
                w_sb[nm] = w
            # Token-quarter loads: the first projection matmuls wait on
            # 1MB of context instead of 4MB.
            c_t = [xc_pool.tile([128, N], F16, tag="c_in", bufs=4,
                                name=f"c_in_{cc}") for cc in range(4)]
            x_t = [xc_pool.tile([128, N], F16, tag="x_in", bufs=4,
                                name=f"x_in_{cc}") for cc in range(4)]
            for q in range(4):
                for cc in range(4):
                    nc.sync.dma_start(out=c_t[cc][:, ts(q, N // 4)],
                                      in_=c_d[ts(cc, 128), ts(q, N // 4)])
            for q in range(4):
                for cc in range(4):
                    nc.gpsimd.dma_start(out=x_t[cc][:, ts(q, N // 4)],
                                        in_=x_d[ts(cc, 128), ts(q, N // 4)])

            # Pre-warm the PE clock during the DMA lead-in: ~5us of dummy
            # matmuls un-throttle HAM (4/8 -> 8/8) so the projections run
            # at 2.4 GHz from their first instruction.  One accumulation
            # group, so Tile adds no inter-matmul semaphores.
            NWARM = 150
            warm_ps = sc_ps.tile([128, 128], F32, tag="sc", name="warm_ps")
            for wi in range(NWARM):
                nc.tensor.matmul(warm_ps, lhsT=ident, rhs=ident,
                                 start=(wi == 0), stop=(wi == NWARM - 1))

            def project(wname, src, dst):
                w = w_sb[wname]
                for half in range(N // 1024):  # 4 slots of 2 n-tiles
                    ps = sc_ps.tile([128, 1024], F32, tag="sc",
                                    name=f"pj_{wname}_{half}")
                    for sub in range(2):
                        t = 2 * half + sub
                        for cc in range(4):
                            nc.tensor.matmul(
                                ps[:, ts(sub, NI)],
                                lhsT=w[:, cc, :], rhs=src[cc][:, ts(t, NI)],
                                start=(cc == 0), stop=(cc == 3),
                            )
                    copy_rr(out=dst[:, ts(half, 1024)], in_=ps)

            project("wkt", c_t, k_sb)
            project("wqt", x_t, q_sb)
            project("wvt", c_t, v_sb)

        # V transposes: vT[j-chunk] = v[:, chunk].T, PE transpose via ident.
        for ch in range(N_JC):
            tp = sc_ps.tile([128, 128], F16, tag="sc", name=f"vtp_{ch}")
            nc.tensor.transpose(tp, v_sb[:, ts(ch, 128)], ident)
            copy_rr(
                out=vT_sb[:, ch, :, 0:HEAD_DIM],
                in_=tp.rearrange("p (h d) -> p h d", h=4),
            )

        # ---------------- Phase 2: attention ----------------
        at_pool = stk.enter_context(tc.tile_pool(name="attn", bufs=1))

        es_ref = {}      # (ib, j) -> es pair-tile list [pair01, pair23]
        acc_ref = {}     # ib -> (accA, accB)
        raw_ref = {}     # ib -> raw tile
        pending_norm = []  # per-head normalize closures, drained 1/chunk
        norm_left = {}   # ib -> heads still to normalize
        pending_op = []  # i-blocks awaiting output projection

        def emit_scores(ib, j):
            """4 row-tiled score MMs -> 2 psum pair-tiles; exp to SBUF."""
            pairs = []
            for p in range(2):  # heads (2p, 2p+1)
                ps = sc_ps.tile([128, 1024], F32, tag="sc",
                                name=f"scps_{ib}_{j}_{p}")
                es = at_pool.tile([128, 1024], F16, tag=f"es{p}", bufs=9,
                                  name=f"es_{ib}_{j}_{p}")
                for hh in range(2):
                    h = 2 * p + hh
                    base = 32 * h
                    nc.tensor.matmul(
                        ps[:, ts(hh, NI)],
                        lhsT=k_sb[base:base + 32, ts(j, NJ)],
                        rhs=q_sb[base:base + 32, ts(ib, NI)],
                        start=True, stop=True,
                        tile_position=(base, 0),
                    )
                eng = pick_exp_engine()
                if eng == "A":
                    nc.scalar.activation(
                        out=es, in_=ps,
                        func=mybir.ActivationFunctionType.Exp,
                    )
                else:
                    nc.vector.tensor_scalar(
                        out=es.bitcast(I16), in0=ps,
                        scalar1=EXP_A, scalar2=EXP_B,
                        op0=mybir.AluOpType.mult,
                        op1=mybir.AluOpType.add,
                    )
                pairs.append(es)
            es_ref[(ib, j)] = pairs

        def emit_av(ib, j):
            if j == 0:
                accA = av_ps.tile([128, NI], F32, tag="accA",
                                  name=f"accA_{ib}")
                accB = av_ps.tile([128, NI], F32, tag="accB",
                                  name=f"accB_{ib}")
                acc_ref[ib] = (accA, accB)
            accA, accB = acc_ref[ib]
            pairs = es_ref.pop((ib, j))
            for p in range(2):
                acc = (accA, accB)[p]
                es = pairs[p]
                for hh in range(2):
                    nc.tensor.matmul(
                        acc[64 * hh:64 * hh + HEAD_DIM + 1, :],
                        lhsT=vT_sb[:, j, 2 * p + hh, :],
                        rhs=es[:, ts(hh, NI)],
                        start=(j == 0), stop=(j == N_JC - 1),
                        tile_position=(0, 64 * hh),
                        skip_group_check=True,
                    )
            if j == N_JC - 1:
                emit_normalize(ib)

        def emit_normalize(ib):
            # Free the acc banks quickly (2 ACT copies), then hand the
            # per-head reciprocal-normalize to GpSimd (DVE only does the
            # tiny reciprocal).  The per-head tails are drained one per
            # chunk by the main loop so neither exp engine sees a burst.
            accA, accB = acc_ref.pop(ib)
            raw = at_pool.tile([128, NI], F16, tag="raw", bufs=2,
                               name=f"raw_{ib}")
            raw_ref[ib] = raw
            norm_left[ib] = 4
            nsbs = []
            for p, acc in enumerate((accA, accB)):
                nsb = at_pool.tile([128, NI], F32, tag=f"nsb{p}", bufs=2,
                                   name=f"nsb_{ib}_{p}")
                # Split across engines so the acc banks free ASAP (the
                # next block's first AV matmul waits on them).
                if eng_t["D"] + 0.6 <= eng_t["A"] + 0.72:
                    nc.vector.tensor_copy(out=nsb, in_=acc)
                    eng_t["D"] += 0.6
                else:
                    nc.scalar.copy(out=nsb, in_=acc)
                    eng_t["A"] += 0.72
                nsbs.append(nsb)

            def make_tail(h):
                p, hh = divmod(h, 2)
                nsb = nsbs[p]

                def tail():
                    lr = at_pool.tile([1, NI], F32, tag="lr", bufs=4,
                                      name=f"lr_{ib}_{h}")
                    rc = at_pool.tile([1, NI], F32, tag="rc", bufs=4,
                                      name=f"rc_{ib}_{h}")
                    bc = at_pool.tile([128, NI], F32, tag="bc", bufs=4,
                                      name=f"bc_{ib}_{h}")
                    nc.vector.tensor_copy(
                        out=lr,
                        in_=nsb[64 * hh + HEAD_DIM:64 * hh + HEAD_DIM + 1, :])
                    nc.vector.reciprocal_approx_fast(out=rc, in_=lr)
                    eng_t["D"] += 0.6
                    # Full-partition broadcast so the SB*SB multiply sees
                    # equal base partitions on both inputs.  The multiply
                    # stays on DVE: mixing gpsimd op types (broadcast +
                    # mul) thrashes its microcode library (~6us per swap).
                    nc.gpsimd.partition_broadcast(bc, rc)
                    nc.vector.tensor_mul(
                        out=raw[ts(h, 32), :],
                        in0=nsb[64 * hh:64 * hh + 32, :],
                        in1=bc[64 * hh:64 * hh + 32, :],
                    )
                    eng_t["D"] += 0.6
                    norm_left[ib] -= 1
                    if norm_left[ib] == 0:
                        del norm_left[ib]
                        pending_op.append(ib)

                return tail

            for h in range(4):
                pending_norm.append(make_tail(h))

        def emit_outproj():
            while pending_op:
                oi = pending_op.pop(0)
                raw = raw_ref.pop(oi)
                for half in range(2):  # cc pairs (0,1) and (2,3)
                    ps = sc_ps.tile([128, 1024], F32, tag="sc",
                                    name=f"ofps_{oi}_{half}")
                    for sub in range(2):
                        cc = 2 * half + sub
                        nc.tensor.matmul(
                            ps[:, ts(sub, NI)],
                            lhsT=wot_sb[:, ts(cc, 128)], rhs=raw,
                            start=True, stop=True,
                        )
                    ot = at_pool.tile([128, 1024], F32, tag="ot", bufs=3,
                                      name=f"ot_{oi}_{half}")
                    copy_rr(out=ot, in_=ps)
                    eng_t["A" if _cp_state["n"] % 2 == 0 else "D"] += 1.2
                    for sub in range(2):
                        cc = 2 * half + sub
                        nc.sync.dma_start(
                            out=y_d[ts(cc, 128), ts(oi, NI)],
                            in_=ot[:, ts(sub, NI)])

        # Main pipeline: score cursor leads the AV cursor by AV_LAG chunks.
        total = N_IB * N_JC
        av_cur = 0
        for s_cur in range(total):
            ib, j = divmod(s_cur, N_JC)
            emit_scores(ib, j)
            # One normalize tail every 4th chunk: keeps the DVE's extra
            # duty ~0.3us/chunk so AV's exps never queue behind it.  In
            # the last block drain every other chunk so the final output
            # projection isn't pushed past the end of the score stream.
            if pending_norm and s_cur % (2 if ib == N_IB - 1 else 4) == 1:
                pending_norm.pop(0)()
            if pending_op:
                emit_outproj()
            navail = 0
            while av_cur <= s_cur - AV_LAG and navail < 2:
                av_ib, av_j = divmod(av_cur, N_JC)
                # Delay each block's first AV a few extra chunks so the
                # PE never queues behind the normalize copies that free
                # the acc banks (keeps HAM warm across block boundaries).
                if av_j == 0 and s_cur < av_ib * N_JC + 4:
                    break
                emit_av(av_ib, av_j)
                av_cur += 1
                navail += 1
        while av_cur < total:
            emit_av(*divmod(av_cur, N_JC))
            av_cur += 1
        while pending_norm:
            pending_norm.pop(0)()
        emit_outproj()

    nc.compile()
    return nc


_NC_CACHE = None


def _get_nc():
    global _NC_CACHE
    if _NC_CACHE is None:
        _NC_CACHE = build_nc()
    return _NC_CACHE


def _shard_inputs(query, context, Wq, Wk, Wv, Wo):
    query = np.asarray(query, dtype=np.float32)
    context = np.asarray(context, dtype=np.float32)
    Wq = np.asarray(Wq, dtype=np.float32)
    Wk = np.asarray(Wk, dtype=np.float32)
    Wv = np.asarray(Wv, dtype=np.float32)
    Wo = np.asarray(Wo, dtype=np.float32)
    b = query.shape[0]
    in_maps = []
    for core in range(N_CORES):
        bb, p = divmod(core, 2)
        sl = slice(128 * p, 128 * (p + 1))
        in_maps.append({
            "x": np.ascontiguousarray(query[bb].reshape(DIM, N).astype(np.float16)),
            "c": np.ascontiguousarray(context[bb].reshape(DIM, N).astype(np.float16)),
            "wqt": np.ascontiguousarray((Wq[sl, :] * SCALE).T.astype(np.float16)),
            "wkt": np.ascontiguousarray(Wk[sl, :].T.astype(np.float16)),
            "wvt": np.ascontiguousarray(Wv[sl, :].T.astype(np.float16)),
            "wot": np.ascontiguousarray(Wo[:, sl].T.astype(np.float16)),
        })
    return in_maps, b


def _run(inputs, trace=False, **kw):
    in_maps, b = _shard_inputs(**inputs)
    nc = _get_nc()
    res = run_bass_kernel_spmd(nc, in_maps, core_ids=list(range(N_CORES)),
                               trace=trace, **kw)
    outs = []
    for bb in range(b):
        y = res.results[2 * bb]["y"] + res.results[2 * bb + 1]["y"]
        outs.append(y.reshape(DIM, 64, 64))
    return np.stack(outs).astype(np.float32), res


def kernel(**inputs):
    out, _ = _run(inputs)
    return out


# revision 30
# speedup vs baseline: 1.1814x; 1.0016x over previous
"""Trainium2 Bass kernel for nn_Attention (dense transformer spatial attention).

Reference computation (per batch b):
    q = Wq @ x   (1x1 conv over channels), k = Wk @ c, v = Wv @ c
    per head h (8 heads, head_dim 32, n = 64*64 = 4096 tokens):
        S = (q_h^T k_h) * DIM**-0.5 ; P = softmax(S, axis=-1) ; o_h = v_h P^T
    out = Wo @ concat(o_h)

Sharding (8 cores): core c handles batch b = c//2 and heads 4*(c%2) .. +4
(tensor-parallel over heads via weight row/col slicing).  The two cores of a
batch produce partial outputs Y = Wo_slice @ o_slice which the host sums.

v2 design (vs the 565us baseline): the kernel is engine-bound on the softmax
exp (every score element must cross ACT/DVE once at 1 elem/cycle/lane fp32),
so the structure keeps both exp engines saturated and the PE dense:

  - i-blocks of 512 tokens (8 blocks), j-chunks of 128 keys (32/block).
  - Scores: per chunk, 4 row-tiled matmuls (K=32 per head at PE rows
    32h) run concurrently, writing two [128, 1024] fp32 PSUM pair-tiles
    (2 heads side by side, 2 banks each) from a 3-deep ring (6 banks).
    One wide exp op per pair-tile (ACT table exp or DVE Schraudolph
    bit-trick), deficit-scheduled across the two engines.
  - AV: lags 2 chunks behind scores; per chunk 4 col-tiled matmuls
    (M=33: 32 v-dims + ones column for the softmax denominator) at PE
    cols 0/64, accumulating into per-head-pair PSUM tiles (2 banks).
  - Normalize: ACT copies acc->SBUF, DVE reciprocal of the denominator
    rows, GpSimd broadcast + multiply (keeps the exp engines free).
  - Output projection rides the score PSUM ring (2 matmuls per
    pair-tile slot), evacuated by ACT/DVE and DMA'd out.
  - The steady chunk cadence keeps PE gaps well under the ~3.4us HAM
    window, so the PE clock stays at 2.4 GHz (the baseline oscillated,
    spending 54% of its time throttled at 1.2 GHz).
"""

import os
import sys

import numpy as np

for _p in ("/opt/trn_rl_repo", "/root/.axon_site/_ro/trn_rl_repo"):
    if os.path.isdir(_p) and _p not in sys.path:
        sys.path.insert(0, _p)

import concourse.bass as bass
import concourse.tile as tile
from concourse import bacc, mybir
from concourse.bass import ts
from concourse.bass_utils import run_bass_kernel_spmd
from concourse.masks import make_identity

DIM = 512
HEAD = 8
ATTN_DIM = 256
HEAD_DIM = 32
N = 4096  # 64 * 64 tokens
SCALE = DIM ** -0.5

N_CORES = 8
NI = 512   # i-block (query tokens per block)
NJ = 128   # j-chunk (key tokens per score matmul)
N_IB = N // NI   # 8 i-blocks
N_JC = N // NJ   # 32 j-chunks
AV_LAG = 5       # AV trails the score cursor by this many chunks.  Deep
                 # enough that a standing queue of ready AV chunks spans
                 # each block boundary, so the acc-bank handoff never
                 # leaves the PE with score-only slots (HAM idle window).

F32 = mybir.dt.float32
F16 = mybir.dt.float16
I16 = mybir.dt.int16

# Schraudolph fast-exp on the Vector engine: the bit pattern of fp16
# exp(x) is approximately int16(x * 1024/ln2 + 15*1024 + sigma).  One
# tensor_scalar (mult, add) with an int16 output view computes it in a
# single instruction; rel err ~N(0, 1.8%) which washes out over the
# 4096-way diffuse softmax.
EXP_A = 1024.0 / float(np.log(2.0))
EXP_B = 15.0 * 1024.0 - 60.0

# Deficit scheduler estimates (us per [128, 1024] exp op, trace-measured).
DVE_EXP_US = 1.222
ACT_EXP_US = 1.072


def build_nc():
    nc = bacc.Bacc()

    x_d = nc.dram_tensor("x", [DIM, N], F16, kind="ExternalInput").ap()
    c_d = nc.dram_tensor("c", [DIM, N], F16, kind="ExternalInput").ap()
    wqt_d = nc.dram_tensor("wqt", [DIM, 128], F16, kind="ExternalInput").ap()
    wkt_d = nc.dram_tensor("wkt", [DIM, 128], F16, kind="ExternalInput").ap()
    wvt_d = nc.dram_tensor("wvt", [DIM, 128], F16, kind="ExternalInput").ap()
    wot_d = nc.dram_tensor("wot", [128, DIM], F16, kind="ExternalInput").ap()
    y_d = nc.dram_tensor("y", [DIM, N], F32, kind="ExternalOutput").ap()

    from contextlib import ExitStack

    with tile.TileContext(nc) as tc, ExitStack() as stk:
        persist = stk.enter_context(tc.tile_pool(name="persist", bufs=1))

        q_sb = persist.tile([128, N], F16)
        k_sb = persist.tile([128, N], F16)
        # vT: (token-in-chunk, j_chunk, head, 32 dims + ones col)
        vT_sb = persist.tile([128, N_JC, 4, HEAD_DIM + 1], F16)
        wot_sb = persist.tile([128, DIM], F16)
        ident = persist.tile([128, 128], F16)

        nc.sync.dma_start(out=wot_sb, in_=wot_d)
        make_identity(nc, ident)
        nc.vector.memset(vT_sb[:, :, :, HEAD_DIM:], 1.0)

        # Preload the exp activation table set during the DMA lead-in so
        # the first real exp doesn't pay the ~2.7us ACT_TABLE_LOAD.
        warm_sb = persist.tile([1, 32], F32)
        nc.vector.memset(warm_sb, 0.0)
        nc.scalar.activation(out=warm_sb, in_=warm_sb,
                             func=mybir.ActivationFunctionType.Exp)

        # Round-robin small PSUM->SBUF evacuations over Vector and Scalar.
        _cp_state = {"n": 0}

        def copy_rr(out, in_):
            _cp_state["n"] += 1
            if _cp_state["n"] % 2:
                nc.vector.tensor_copy(out=out, in_=in_)
            else:
                nc.scalar.copy(out=out, in_=in_)

        # Exp-engine deficit scheduler (us of queued work per engine).
        eng_t = {"D": 0.0, "A": 0.0}

        def pick_exp_engine():
            if eng_t["D"] + DVE_EXP_US <= eng_t["A"] + ACT_EXP_US:
                eng_t["D"] += DVE_EXP_US
                return "D"
            eng_t["A"] += ACT_EXP_US
            return "A"

        # ---------------- Phase 1: projections ----------------
        cw_pool = stk.enter_context(tc.tile_pool(name="cw", bufs=1))
        v_sb = cw_pool.tile([128, N], F16, tag="v_sb")

        # Score/proj/outproj PSUM ring: 3 x [128, 1024] fp32 (6 banks).
        sc_ps = stk.enter_context(tc.tile_pool(name="sc_ps", bufs=3,
                                               space="PSUM"))
        # AV accumulators: heads (0,1) and (2,3), [*, 512] fp32 (2 banks).
        av_ps = stk.enter_context(tc.tile_pool(name="av_ps", bufs=1,
                                               space="PSUM"))

        with tc.tile_pool(name="xc_in", bufs=1) as xc_pool:
            w_sb = {}
            for nm, d in (("wkt", wkt_d), ("wqt", wqt_d), ("wvt", wvt_d)):
                w = xc_pool.tile([128, 4, 128], F16, tag=nm)
                nc.sync.dma_start(out=w, in_=d.rearrange("(c p) m -> p c m", p=128))
                w_sb[nm] = w
            # Token-quarter loads: the first projection matmuls wait on
            # 1MB of context instead of 4MB.
            c_t = [xc_pool.tile([128, N], F16, tag="c_in", bufs=4,
                                name=f"c_in_{cc}") for cc in range(4)]
            x_t = [xc_pool.tile([128, N], F16, tag="x_in", bufs=4,
                                name=f"x_in_{cc}") for cc in range(4)]
            for q in range(4):
                for cc in range(4):
                    nc.sync.dma_start(out=c_t[cc][:, ts(q, N // 4)],
                                      in_=c_d[ts(cc, 128), ts(q, N // 4)])
            for q in range(4):
                for cc in range(4):
                    nc.gpsimd.dma_start(out=x_t[cc][:, ts(q, N // 4)],
                                        in_=x_d[ts(cc, 128), ts(q, N // 4)])

            # Pre-warm the PE clock during the DMA lead-in: ~5us of dummy
            # matmuls un-throttle HAM (4/8 -> 8/8) so the projections run
            # at 2.4 GHz from their first instruction.  One accumulation
            # group, so Tile adds no inter-matmul semaphores.
            NWARM = 150
            warm_ps = sc_ps.tile([128, 128], F32, tag="sc", name="warm_ps")
            for wi in range(NWARM):
                nc.tensor.matmul(warm_ps, lhsT=ident, rhs=ident,
                                 start=(wi == 0), stop=(wi == NWARM - 1))

            def project(wname, src, dst):
                w = w_sb[wname]
                for half in range(N // 1024):  # 4 slots of 2 n-tiles
                    ps = sc_ps.tile([128, 1024], F32, tag="sc",
                                    name=f"pj_{wname}_{half}")
                    for sub in range(2):
                        t = 2 * half + sub
                        for cc in range(4):
                            nc.tensor.matmul(
                                ps[:, ts(sub, NI)],
                                lhsT=w[:, cc, :], rhs=src[cc][:, ts(t, NI)],
                                start=(cc == 0), stop=(cc == 3),
                            )
                    copy_rr(out=dst[:, ts(half, 1024)], in_=ps)

            project("wkt", c_t, k_sb)
            project("wqt", x_t, q_sb)
            project("wvt", c_t, v_sb)

        # V transposes: vT[j-chunk] = v[:, chunk].T, PE transpose via ident.
        for ch in range(N_JC):
            tp = sc_ps.tile([128, 128], F16, tag="sc", name=f"vtp_{ch}")
            nc.tensor.transpose(tp, v_sb[:, ts(ch, 128)], ident)
            copy_rr(
                out=vT_sb[:, ch, :, 0:HEAD_DIM],
                in_=tp.rearrange("p (h d) -> p h d", h=4),
            )

        # ---------------- Phase 2: attention ----------------
        at_pool = stk.enter_context(tc.tile_pool(name="attn", bufs=1))

        es_ref = {}      # (ib, j) -> es pair-tile list [pair01, pair23]
        acc_ref = {}     # ib -> (accA, accB)
        raw_ref = {}     # ib -> raw tile
        pending_norm = []  # per-head normalize closures, drained 1/chunk
        norm_left = {}   # ib -> heads still to normalize
        pending_op = []  # i-blocks awaiting output projection

        def emit_scores(ib, j):
            """4 row-tiled score MMs -> 2 psum pair-tiles; exp to SBUF."""
            pairs = []
            for p in range(2):  # heads (2p, 2p+1)
                ps = sc_ps.tile([128, 1024], F32, tag="sc",
                                name=f"scps_{ib}_{j}_{p}")
                es = at_pool.tile([128, 1024], F16, tag=f"es{p}", bufs=9,
                                  name=f"es_{ib}_{j}_{p}")
                for hh in range(2):
                    h = 2 * p + hh
                    base = 32 * h
                    nc.tensor.matmul(
                        ps[:, ts(hh, NI)],
                        lhsT=k_sb[base:base + 32, ts(j, NJ)],
                        rhs=q_sb[base:base + 32, ts(ib, NI)],
                        start=True, stop=True,
                        tile_position=(base, 0),
                    )
                eng = pick_exp_engine()
                if eng == "A":
                    nc.scalar.activation(
                        out=es, in_=ps,
                        func=mybir.ActivationFunctionType.Exp,
                    )
                else:
                    nc.vector.tensor_scalar(
                        out=es.bitcast(I16), in0=ps,
                        scalar1=EXP_A, scalar2=EXP_B,
                        op0=mybir.AluOpType.mult,
                        op1=mybir.AluOpType.add,
                    )
                pairs.append(es)
            es_ref[(ib, j)] = pairs

        def emit_av(ib, j):
            if j == 0:
                accA = av_ps.tile([128, NI], F32, tag="accA",
                                  name=f"accA_{ib}")
                accB = av_ps.tile([128, NI], F32, tag="accB",
                                  name=f"accB_{ib}")
                acc_ref[ib] = (accA, accB)
            accA, accB = acc_ref[ib]
            pairs = es_ref.pop((ib, j))
            for p in range(2):
                acc = (accA, accB)[p]
                es = pairs[p]
                for hh in range(2):
                    nc.tensor.matmul(
                        acc[64 * hh:64 * hh + HEAD_DIM + 1, :],
                        lhsT=vT_sb[:, j, 2 * p + hh, :],
                        rhs=es[:, ts(hh, NI)],
                        start=(j == 0), stop=(j == N_JC - 1),
                        tile_position=(0, 64 * hh),
                        skip_group_check=True,
                    )
            if j == N_JC - 1:
                emit_normalize(ib)

        def emit_normalize(ib):
            # Free the acc banks quickly (2 ACT copies), then hand the
            # per-head reciprocal-normalize to GpSimd (DVE only does the
            # tiny reciprocal).  The per-head tails are drained one per
            # chunk by the main loop so neither exp engine sees a burst.
            accA, accB = acc_ref.pop(ib)
            raw = at_pool.tile([128, NI], F16, tag="raw", bufs=2,
                               name=f"raw_{ib}")
            raw_ref[ib] = raw
            norm_left[ib] = 4
            nsbs = []
            for p, acc in enumerate((accA, accB)):
                nsb = at_pool.tile([128, NI], F32, tag=f"nsb{p}", bufs=2,
                                   name=f"nsb_{ib}_{p}")
                # Split across engines so the acc banks free ASAP (the
                # next block's first AV matmul waits on them).
                if eng_t["D"] + 0.6 <= eng_t["A"] + 0.72:
                    nc.vector.tensor_copy(out=nsb, in_=acc)
                    eng_t["D"] += 0.6
                else:
                    nc.scalar.copy(out=nsb, in_=acc)
                    eng_t["A"] += 0.72
                nsbs.append(nsb)

            def make_tail(h):
                p, hh = divmod(h, 2)
                nsb = nsbs[p]

                def tail():
                    lr = at_pool.tile([1, NI], F32, tag="lr", bufs=4,
                                      name=f"lr_{ib}_{h}")
                    rc = at_pool.tile([1, NI], F32, tag="rc", bufs=4,
                                      name=f"rc_{ib}_{h}")
                    bc = at_pool.tile([128, NI], F32, tag="bc", bufs=4,
                                      name=f"bc_{ib}_{h}")
                    # lr staging on ACT: DVE is the busier exp engine.
                    nc.scalar.copy(
                        out=lr,
                        in_=nsb[64 * hh + HEAD_DIM:64 * hh + HEAD_DIM + 1, :])
                    eng_t["A"] += 0.72
                    nc.vector.reciprocal_approx_fast(out=rc, in_=lr)
                    eng_t["D"] += 0.6
                    # Full-partition broadcast so the SB*SB multiply sees
                    # equal base partitions on both inputs.  The multiply
                    # stays on DVE: mixing gpsimd op types (broadcast +
                    # mul) thrashes its microcode library (~6us per swap).
                    nc.gpsimd.partition_broadcast(bc, rc)
                    nc.vector.tensor_mul(
                        out=raw[ts(h, 32), :],
                        in0=nsb[64 * hh:64 * hh + 32, :],
                        in1=bc[64 * hh:64 * hh + 32, :],
                    )
                    eng_t["D"] += 0.6
                    norm_left[ib] -= 1
                    if norm_left[ib] == 0:
                        del norm_left[ib]
                        pending_op.append(ib)

                return tail

            for h in range(4):
                pending_norm.append(make_tail(h))

        def emit_outproj():
            while pending_op:
                oi = pending_op.pop(0)
                raw = raw_ref.pop(oi)
                for half in range(2):  # cc pairs (0,1) and (2,3)
                    ps = sc_ps.tile([128, 1024], F32, tag="sc",
                                    name=f"ofps_{oi}_{half}")
                    for sub in range(2):
                        cc = 2 * half + sub
                        nc.tensor.matmul(
                            ps[:, ts(sub, NI)],
                            lhsT=wot_sb[:, ts(cc, 128)], rhs=raw,
                            start=True, stop=True,
                        )
                    ot = at_pool.tile([128, 1024], F32, tag="ot", bufs=3,
                                      name=f"ot_{oi}_{half}")
                    # Deficit-scheduled evacuation (DVE is usually busier).
                    if eng_t["D"] + 1.25 <= eng_t["A"] + 1.15:
                        nc.vector.tensor_copy(out=ot, in_=ps)
                        eng_t["D"] += 1.25
                    else:
                        nc.scalar.copy(out=ot, in_=ps)
                        eng_t["A"] += 1.15
                    for sub in range(2):
                        cc = 2 * half + sub
                        nc.sync.dma_start(
                            out=y_d[ts(cc, 128), ts(oi, NI)],
                            in_=ot[:, ts(sub, NI)])

        # Main pipeline: score cursor leads the AV cursor by AV_LAG chunks.
        total = N_IB * N_JC
        av_cur = 0
        for s_cur in range(total):
            ib, j = divmod(s_cur, N_JC)
            emit_scores(ib, j)
            # One normalize tail every 4th chunk: keeps the DVE's extra
            # duty ~0.3us/chunk so AV's exps never queue behind it.  In
            # the last block drain every other chunk so the final output
            # projection isn't pushed past the end of the score stream.
            if pending_norm and s_cur % (2 if ib == N_IB - 1 else 4) == 1:
                pending_norm.pop(0)()
            if pending_op:
                emit_outproj()
            navail = 0
            while av_cur <= s_cur - AV_LAG and navail < 2:
                av_ib, av_j = divmod(av_cur, N_JC)
                # Delay each block's first AV a few extra chunks so the
                # PE never queues behind the normalize copies that free
                # the acc banks (keeps HAM warm across block boundaries).
                if av_j == 0 and s_cur < av_ib * N_JC + 4:
                    break
                emit_av(av_ib, av_j)
                av_cur += 1
                navail += 1
        while av_cur < total:
            emit_av(*divmod(av_cur, N_JC))
            av_cur += 1
        while pending_norm:
            pending_norm.pop(0)()
        emit_outproj()

    nc.compile()
    return nc


_NC_CACHE = None


def _get_nc():
    global _NC_CACHE
    if _NC_CACHE is None:
        _NC_CACHE = build_nc()
    return _NC_CACHE


def _shard_inputs(query, context, Wq, Wk, Wv, Wo):
    query = np.asarray(query, dtype=np.float32)
    context = np.asarray(context, dtype=np.float32)
    Wq = np.asarray(Wq, dtype=np.float32)
    Wk = np.asarray(Wk, dtype=np.float32)
    Wv = np.asarray(Wv, dtype=np.float32)
    Wo = np.asarray(Wo, dtype=np.float32)
    b = query.shape[0]
    in_maps = []
    for core in range(N_CORES):
        bb, p = divmod(core, 2)
        sl = slice(128 * p, 128 * (p + 1))
        in_maps.append({
            "x": np.ascontiguousarray(query[bb].reshape(DIM, N).astype(np.float16)),
            "c": np.ascontiguousarray(context[bb].reshape(DIM, N).astype(np.float16)),
            "wqt": np.ascontiguousarray((Wq[sl, :] * SCALE).T.astype(np.float16)),
            "wkt": np.ascontiguousarray(Wk[sl, :].T.astype(np.float16)),
            "wvt": np.ascontiguousarray(Wv[sl, :].T.astype(np.float16)),
            "wot": np.ascontiguousarray(Wo[:, sl].T.astype(np.float16)),
        })
    return in_maps, b


def _run(inputs, trace=False, **kw):
    in_maps, b = _shard_inputs(**inputs)
    nc = _get_nc()
    res = run_bass_kernel_spmd(nc, in_maps, core_ids=list(range(N_CORES)),
                               trace=trace, **kw)
    outs = []
    for bb in range(b):
        y = res.results[2 * bb]["y"] + res.results[2 * bb + 1]["y"]
        outs.append(y.reshape(DIM, 64, 64))
    return np.stack(outs).astype(np.float32), res


def kernel(**inputs):
    out, _ = _run(inputs)
    return out
